# revision 26
# baseline (speedup 1.0000x reference)
"""CrissCrossAttention Trainium2 kernel.

Per-core: one batch b of x [C=512, HW=9216] (h-major pixels, p = h*96+w).

Math (reference):
  q = Wq x + bq ; k = Wk x + bk ; v = Wv x + bv        (1x1 convs)
  E_col[g,h] per w = sum_c k[c,g,w] q[c,h,w]  (diag g==h masked -inf)
  E_row[v,w] per h                                      (row logits)
  attn = softmax over concat(H' + W') per dest pixel
  out = gamma*(out_h + out_w) + x

The end-to-end time in this axon-tunneled setup is dominated by the
host<->device wire (~40-55MB/s per process, up+down combined), so the
design minimizes bytes on the wire and overlaps host work / upload / exec
/ download.  Wire budget per batch: 5.0MB up + 3.6MB down = 69MB total
(vs 11.8MB/batch for the naive int8-x + fp16-qk + int8-out layout).

Host (single CPU core):
  - x' = x + gamma*bv (residual shift folding bv; v-path correction row
    -Wv(gamma*bv) added on device via K=1 matmul; bq/bk adjusted inside the
    correction stream).
  - x' shipped as per-channel int7 (8 values packed in 7 bytes) + fp32
    steps (4.0MB/batch).
  - q/k are NOT shipped.  The device computes qk = Wqk @ xhat in fp32 on
    the PE, where xhat is the int7-dequantized x'.  The host ships a small
    int6-packed correction stream cq = quant(Wqk @ (x' - xhat) + bqk)
    [2*IC, HW] (0.9MB/batch).  Host and device both evaluate Wqk @ xhat in
    fp32, so adding the correction reconstructs q/k to ~fp32 accuracy at
    ~1/3 the bytes of fp16 q/k shipping.  Host pays one [128,512]x[512,9216]
    sgemm on the quantization residual (same flops as shipping q/k would).
  - residual add and the output-delta dequant happen on host in fp32.
  - measured rel err 1.2e-2 (gate 2e-2); error sources: int7 x (v-path),
    int6 output delta, bf16 P/v/U, each validated against a numpy
    simulation of the exact quantization pipeline.

Device (per core, Tile framework):
  - Phase P: unpack int7 x' (shift/and/or vector ops) -> fp32 x32 and bf16
    xbb; q/k = fp32 PE matmuls of wqkT x32 + unpacked int6 correction
    (offset -32*cs folded in via K=1 matmul); v = Wv xbb via bf16 matmuls.
  - P = exp(logits) unnormalized bf16 (values up to e^40 overflow fp16);
    denominators D = colsum + rowsum via ones-matmuls; Rg = gamma/D.
  - U_colT(w) / U_rowT(h) -> [96, C] bf16 scratch in DRAM; final pass
    DMA-transposes them back, sums to delta = gamma*(out_h+out_w), and
    emits delta as int6 (4 values packed into 3 bytes by shift/or vector
    ops) with per-channel per-512px-block fp32 amax [C, 18].

Dispatch: replicates run_bass_kernel_spmd's axon path (shard_map +
_bass_exec_p custom call) but builds the jitted executables ONCE and caches
them; donated zero output buffers are created on-device (no host upload);
the 8 cores run as 4 groups of 2 so later groups' upload/exec overlap
earlier groups' downloads, whose fetches start eagerly at dispatch time
(up+down duplex measures faster than either direction alone).  On failure
the kernel falls back to a full single-process rebuild and finally to
run_bass_kernel_spmd.  A dual-process mode (second lightweight PJRT
client owning the last KERNEL_WCORES=2 cores, shared-memory handoff)
exists behind KERNEL_DUAL=1: a second client adds a second serialized
D2H stream (downloads cap ~25MB/s per client) and nearly doubles
pure-transfer benchmarks, but every wire byte costs host CPU and the one
host core pumps ~45-50MB/s total across processes, so end-to-end it
measures no faster and stays off by default.
"""

import numpy as np
import ml_dtypes
from concurrent.futures import ThreadPoolExecutor

C, IC, H, W = 512, 64, 96, 96
HW = H * W  # 9216
NB = 18  # 512-wide pixel blocks
NCORES = 8
BF = ml_dtypes.bfloat16
QMARGIN = 30.5  # int6 quant margin (|q| <= 31)
PACKW = HW // 4 * 3  # 6912 packed bytes per channel row (int6 out, int6 c)
XPACKW = HW // 8 * 7  # 8064 packed bytes per channel row (int7 x)
OUTW = PACKW + 4 * NB  # outp row: packed int6 delta + amax18 f32 bytes


def _build(gamma_f: float):
    from contextlib import ExitStack
    import concourse.bass as bass
    import concourse.bacc as bacc
    import concourse.tile as tile
    from concourse import mybir

    f32 = mybir.dt.float32
    bf16 = mybir.dt.bfloat16
    i8 = mybir.dt.int8
    u8 = mybir.dt.uint8
    AF = mybir.ActivationFunctionType
    OP = mybir.AluOpType

    nc = bacc.Bacc("TRN2", target_bir_lowering=False, debug=False)

    xp_d = nc.dram_tensor("xp", [C, XPACKW], u8, kind="ExternalInput").ap()
    xs_d = nc.dram_tensor("xs", [128, 4], f32, kind="ExternalInput").ap()
    cp_d = nc.dram_tensor("cp", [2 * IC, PACKW], u8, kind="ExternalInput").ap()
    cs_d = nc.dram_tensor("cs", [IC, 2], f32, kind="ExternalInput").ap()
    csn_d = nc.dram_tensor("csn", [2, 512], f32, kind="ExternalInput").ap()
    wqkT_d = nc.dram_tensor("wqkT", [4, 128, 2 * IC], f32, kind="ExternalInput").ap()
    wv_d = nc.dram_tensor("wvT", [4, 128, C], bf16, kind="ExternalInput").ap()
    mwvd_d = nc.dram_tensor("mwvd", [1, C], bf16, kind="ExternalInput").ap()
    ib_d = nc.dram_tensor("ib", [96, 96], f32, kind="ExternalInput").ap()
    negib_d = nc.dram_tensor("negib", [96, 96], f32, kind="ExternalInput").ap()
    outp_d = nc.dram_tensor("outp", [C, OUTW], u8, kind="ExternalOutput").ap()

    vt_d = nc.dram_tensor("vt_scratch", [HW, C], bf16, kind="Internal").ap()
    uc_d = nc.dram_tensor("uc_scratch", [HW, C], bf16, kind="Internal").ap()
    ur_d = nc.dram_tensor("ur_scratch", [HW, C], bf16, kind="Internal").ap()
    sc_d = nc.dram_tensor("sc_scratch", [1, HW], f32, kind="Internal").ap()
    sr_d = nc.dram_tensor("sr_scratch", [1, HW], f32, kind="Internal").ap()

    with tile.TileContext(nc) as tc, ExitStack() as top:
        const = top.enter_context(tc.tile_pool(name="const", bufs=1))
        persist = top.enter_context(tc.tile_pool(name="persist", bufs=1))

        wv_sb = const.tile([128, 4, C], bf16)
        nc.sync.dma_start(out=wv_sb, in_=wv_d.rearrange("c p m -> p c m"))
        wqkT_sb = const.tile([128, 4, 2 * IC], f32)
        nc.sync.dma_start(out=wqkT_sb, in_=wqkT_d.rearrange("c p m -> p c m"))
        mwvd_sb = const.tile([1, C], bf16)
        nc.sync.dma_start(out=mwvd_sb, in_=mwvd_d)
        ib_sb = const.tile([96, 96], f32)
        nc.sync.dma_start(out=ib_sb, in_=ib_d)
        negib_sb = const.tile([96, 96], f32)
        nc.sync.dma_start(out=negib_sb, in_=negib_d)
        xs_sb = const.tile([128, 4], f32)
        nc.sync.dma_start(out=xs_sb, in_=xs_d)
        cs_sb = const.tile([IC, 2], f32)
        nc.sync.dma_start(out=cs_sb, in_=cs_d)
        csnq_sb = const.tile([1, 512], f32)  # -32*cs_q in cols 0:IC
        nc.sync.dma_start(out=csnq_sb, in_=csn_d[0:1, :])
        csnk_sb = const.tile([1, 512], f32)  # -32*cs_k in cols 0:IC
        nc.sync.dma_start(out=csnk_sb, in_=csn_d[1:2, :])
        ones1_sb = const.tile([1, 128], bf16)
        nc.vector.memset(ones1_sb, 1.0)
        ones96_sb = const.tile([96, 1], bf16)
        nc.vector.memset(ones96_sb, 1.0)
        onesf_sb = const.tile([1, 512], f32)
        nc.vector.memset(onesf_sb, 1.0)
        xoff_sb = const.tile([128, 4], f32)  # -64 * step, for int7 decode
        nc.vector.tensor_scalar_mul(xoff_sb, xs_sb, -64.0)

        q_sb = persist.tile([IC, HW], f32)
        k_sb = persist.tile([IC, HW], f32)
        pc_sb = persist.tile([96, HW], bf16)  # exp(col logits), [g, (w,h)] w-major
        pr_sb = persist.tile([96, HW], bf16)  # exp(row logits), [v, (h,w)] h-major
        rg_sb = persist.tile([96, 96], f32)  # gamma/D, [h, w]
        rgt_sb = persist.tile([96, 96], f32)  # [w, h]

        # ---------- Phase P: unpack+dequant, q/k fp32 projection + c, v ----
        xv = xp_d.rearrange("(cc p) n -> p cc n", p=128)
        vtw = vt_d.rearrange("(q pt p) c -> q p pt c", pt=4, p=128)
        with ExitStack() as ph, tc.tile_pool(name="pstage", bufs=2) as stage, \
                tc.tile_pool(name="qkpsum", bufs=2, space="PSUM") as psqk, \
                tc.tile_pool(name="ppsum", bufs=2, space="PSUM") as psv:
            for nb in range(NB):
                s, e = nb * 512, (nb + 1) * 512
                xpt = stage.tile([128, 4, 448], u8, tag="xp")
                nc.sync.dma_start(out=xpt, in_=xv[:, :, nb * 448:(nb + 1) * 448])
                # int7 unpack: 7 bytes -> 8 values (LE 56-bit words)
                bl = xpt.rearrange("p cc (n seven) -> p cc n seven", seven=7)
                xu = stage.tile([128, 4, 512], u8, tag="xu")
                vl = xu.rearrange("p cc (n eight) -> p cc n eight", eight=8)
                ta = stage.tile([128, 4, 64], u8, tag="ta")
                tb = stage.tile([128, 4, 64], u8, tag="tb")
                nc.vector.tensor_scalar(vl[:, :, :, 0], bl[:, :, :, 0], 127, None,
                                        op0=OP.bitwise_and)
                for i in range(1, 7):
                    # v_i = (b_{i-1} >> (8-i)) | ((b_i & (2^(7-i)-1)) << i)
                    nc.vector.tensor_scalar(ta, bl[:, :, :, i - 1], 8 - i, None,
                                            op0=OP.logical_shift_right)
                    nc.vector.tensor_scalar(tb, bl[:, :, :, i], (1 << (7 - i)) - 1,
                                            i, op0=OP.bitwise_and,
                                            op1=OP.logical_shift_left)
                    nc.vector.tensor_tensor(vl[:, :, :, i], ta, tb, op=OP.bitwise_or)
                nc.vector.tensor_scalar(vl[:, :, :, 7], bl[:, :, :, 6], 1, None,
                                        op0=OP.logical_shift_right)
                # dequant: x32 = xu*step - 64*step
                x32 = stage.tile([128, 4, 512], f32, tag="x32")
                for cc in range(4):
                    if (nb + cc) % 2 == 0:
                        nc.vector.tensor_scalar(x32[:, cc, :], xu[:, cc, :],
                                                xs_sb[:, cc:cc + 1],
                                                xoff_sb[:, cc:cc + 1],
                                                op0=OP.mult, op1=OP.add)
                    else:
                        nc.scalar.activation(x32[:, cc, :], xu[:, cc, :],
                                             AF.Identity,
                                             bias=xoff_sb[:, cc:cc + 1],
                                             scale=xs_sb[:, cc:cc + 1])
                xbb = stage.tile([128, 4, 512], bf16, tag="xbb")
                if nb % 2 == 0:
                    nc.scalar.copy(xbb, x32)
                else:
                    nc.gpsimd.tensor_copy(xbb, x32)
                # int6 correction stream: 3 bytes -> 4 values, offset 32
                cqs = stage.tile([IC, 384], u8, tag="cqs")
                nc.sync.dma_start(out=cqs, in_=cp_d[0:IC, nb * 384:(nb + 1) * 384])
                cks = stage.tile([IC, 384], u8, tag="cks")
                nc.sync.dma_start(out=cks, in_=cp_d[IC:2 * IC, nb * 384:(nb + 1) * 384])
                cuq = stage.tile([IC, 512], u8, tag="cuq")
                cuk = stage.tile([IC, 512], u8, tag="cuk")
                tc1 = stage.tile([IC, 128], u8, tag="tc1")
                tc2 = stage.tile([IC, 128], u8, tag="tc2")
                for csrc, cdst in ((cqs, cuq), (cks, cuk)):
                    b3 = csrc.rearrange("p (n three) -> p n three", three=3)
                    v4 = cdst.rearrange("p (n four) -> p n four", four=4)
                    nc.vector.tensor_scalar(v4[:, :, 0], b3[:, :, 0], 63, None,
                                            op0=OP.bitwise_and)
                    nc.vector.tensor_scalar(tc1, b3[:, :, 0], 6, None,
                                            op0=OP.logical_shift_right)
                    nc.vector.tensor_scalar(tc2, b3[:, :, 1], 15, 2,
                                            op0=OP.bitwise_and,
                                            op1=OP.logical_shift_left)
                    nc.vector.tensor_tensor(v4[:, :, 1], tc1, tc2, op=OP.bitwise_or)
                    nc.vector.tensor_scalar(tc1, b3[:, :, 1], 4, None,
                                            op0=OP.logical_shift_right)
                    nc.vector.tensor_scalar(tc2, b3[:, :, 2], 3, 4,
                                            op0=OP.bitwise_and,
                                            op1=OP.logical_shift_left)
                    nc.vector.tensor_tensor(v4[:, :, 2], tc1, tc2, op=OP.bitwise_or)
                    nc.vector.tensor_scalar(v4[:, :, 3], b3[:, :, 2], 2, None,
                                            op0=OP.logical_shift_right)
                pq = psqk.tile([IC, 512], f32, tag="pq")
                pk = psqk.tile([IC, 512], f32, tag="pk")
                for cc in range(4):
                    nc.tensor.matmul(pq, lhsT=wqkT_sb[:, cc, 0:IC],
                                     rhs=x32[:, cc, :],
                                     start=(cc == 0), stop=False)
                nc.tensor.matmul(pq, lhsT=csnq_sb[:, 0:IC], rhs=onesf_sb,
                                 start=False, stop=True)
                for cc in range(4):
                    nc.tensor.matmul(pk, lhsT=wqkT_sb[:, cc, IC:2 * IC],
                                     rhs=x32[:, cc, :],
                                     start=(cc == 0), stop=False)
                nc.tensor.matmul(pk, lhsT=csnk_sb[:, 0:IC], rhs=onesf_sb,
                                 start=False, stop=True)
                nc.vector.scalar_tensor_tensor(q_sb[:, s:e], cuq,
                                               cs_sb[:, 0:1], pq,
                                               op0=OP.mult, op1=OP.add)
                nc.vector.scalar_tensor_tensor(k_sb[:, s:e], cuk,
                                               cs_sb[:, 1:2], pk,
                                               op0=OP.mult, op1=OP.add)
                # v-path
                vstage = stage.tile([128, 4, 512], bf16, tag="vst")
                for pt in range(4):
                    pv = psv.tile([128, 512], f32, tag="pv")
                    for cc in range(4):
                        nc.tensor.matmul(pv, lhsT=xbb[:, cc, pt * 128:(pt + 1) * 128],
                                         rhs=wv_sb[:, cc, :], start=(cc == 0), stop=False)
                    nc.tensor.matmul(pv, lhsT=ones1_sb, rhs=mwvd_sb, start=False, stop=True)
                    if pt % 2 == 0:
                        nc.scalar.copy(vstage[:, pt, :], pv)
                    else:
                        nc.vector.tensor_copy(vstage[:, pt, :], pv)
                nc.sync.dma_start(out=vtw[nb], in_=vstage)

        # ---------------- Phase L: logits, exp, sums ----------------
        kc = k_sb.rearrange("c (g w) -> c g w", w=96)
        qc = q_sb.rearrange("c (g w) -> c g w", w=96)
        with ExitStack() as ph, tc.tile_pool(name="lpsum", bufs=4, space="PSUM") as pse, \
                tc.tile_pool(name="spsum", bufs=2, space="PSUM") as pss, \
                tc.tile_pool(name="sstage", bufs=2) as sst:
            for hg in range(24):
                pe4 = pse.tile([96, 384], f32, tag="pe")
                for hi in range(4):
                    h = hg * 4 + hi
                    sl = slice(hi * 96, (hi + 1) * 96)
                    nc.tensor.matmul(pe4[:, sl], lhsT=k_sb[:, h * 96:(h + 1) * 96],
                                     rhs=q_sb[:, h * 96:(h + 1) * 96],
                                     start=True, stop=True)
                nc.scalar.activation(pr_sb[:, hg * 384:(hg + 1) * 384], pe4, AF.Exp)
            for wg in range(24):
                pe4 = pse.tile([96, 384], f32, tag="pe")
                for wi in range(4):
                    w = wg * 4 + wi
                    sl = slice(wi * 96, (wi + 1) * 96)
                    nc.tensor.matmul(pe4[:, sl], lhsT=kc[:, :, w], rhs=qc[:, :, w],
                                     start=True, stop=False)
                    nc.tensor.matmul(pe4[:, sl], lhsT=ib_sb, rhs=negib_sb,
                                     start=False, stop=True)
                nc.scalar.activation(pc_sb[:, wg * 384:(wg + 1) * 384], pe4, AF.Exp)
            for j in range(NB):
                s, e = j * 512, (j + 1) * 512
                p1 = pss.tile([1, 512], f32, tag="p1")
                nc.tensor.matmul(p1, lhsT=ones96_sb, rhs=pc_sb[:, s:e], start=True, stop=True)
                t1 = sst.tile([1, 512], f32, tag="t1")
                nc.vector.tensor_copy(t1, p1)
                nc.sync.dma_start(out=sc_d[:, s:e], in_=t1)
                p2 = pss.tile([1, 512], f32, tag="p2")
                nc.tensor.matmul(p2, lhsT=ones96_sb, rhs=pr_sb[:, s:e], start=True, stop=True)
                t2 = sst.tile([1, 512], f32, tag="t2")
                nc.scalar.copy(t2, p2)
                nc.sync.dma_start(out=sr_d[:, s:e], in_=t2)

        # ---------------- Phase D: denominators -> Rg, RgT ----------------
        with ExitStack() as ph, tc.tile_pool(name="dsmall", bufs=1) as dsm, \
                tc.tile_pool(name="dpsum", bufs=1, space="PSUM") as dps:
            sct = dsm.tile([96, 96], f32)  # [w, h]
            nc.sync.dma_start(out=sct, in_=sc_d.rearrange("one (w h) -> (one w) h", h=96))
            srt = dsm.tile([96, 96], f32)  # [h, w]
            nc.sync.dma_start(out=srt, in_=sr_d.rearrange("one (h w) -> (one h) w", w=96))
            ptr = dps.tile([96, 96], f32)
            nc.tensor.transpose(ptr, sct, ib_sb)  # -> [h, w]
            d_sb = dsm.tile([96, 96], f32)
            nc.vector.tensor_add(d_sb, ptr, srt)
            r_sb = dsm.tile([96, 96], f32)
            nc.vector.reciprocal(r_sb, d_sb)
            nc.scalar.activation(rg_sb, r_sb, AF.Copy, scale=float(gamma_f))
            ptr2 = dps.tile([96, 96], f32)
            nc.tensor.transpose(ptr2, rg_sb, ib_sb)
            nc.vector.tensor_copy(rgt_sb, ptr2)

        # ------- Phases C+R interleaved: column + row attention -------
        vtc = vt_d.rearrange("(g wg wi) c -> wg g wi c", wg=24, wi=4)
        ucw = uc_d.rearrange("(h wg wi) c -> wg h wi c", wg=24, wi=4)
        vtr = vt_d.rearrange("(hg hi v) c -> hg v hi c", hg=24, hi=4)
        urw = ur_d.rearrange("(hg hi w) c -> hg w hi c", hg=24, hi=4)
        with ExitStack() as ph, tc.tile_pool(name="crstage", bufs=4) as cst, \
                tc.tile_pool(name="cpsum", bufs=3, space="PSUM") as psu, \
                tc.tile_pool(name="rpsum", bufs=3, space="PSUM") as psr:
            for grp in range(24):
                wg = grp
                vc = cst.tile([96, 4, C], bf16, tag="vc")
                nc.sync.dma_start(out=vc, in_=vtc[wg])
                uc = cst.tile([96, 4, C], bf16, tag="uc")
                for wi in range(4):
                    w = wg * 4 + wi
                    pu = psu.tile([96, C], f32, tag="pu")
                    nc.tensor.matmul(pu, lhsT=pc_sb[:, w * 96:(w + 1) * 96],
                                     rhs=vc[:, wi, :], start=True, stop=True)
                    if w % 2 == 0:
                        nc.scalar.activation(uc[:, wi, :], pu, AF.Copy,
                                             scale=rg_sb[:, w:w + 1])
                    else:
                        nc.vector.tensor_scalar_mul(uc[:, wi, :], pu, rg_sb[:, w:w + 1])
                nc.sync.dma_start(out=ucw[wg], in_=uc)
                hg = grp
                vr = cst.tile([96, 4, C], bf16, tag="vr")
                nc.sync.dma_start(out=vr, in_=vtr[hg])
                ur = cst.tile([96, 4, C], bf16, tag="ur")
                for hi in range(4):
                    h = hg * 4 + hi
                    pu = psr.tile([96, C], f32, tag="pur")
                    nc.tensor.matmul(pu, lhsT=pr_sb[:, h * 96:(h + 1) * 96],
                                     rhs=vr[:, hi, :], start=True, stop=True)
                    if h % 2 == 0:
                        nc.scalar.activation(ur[:, hi, :], pu, AF.Copy,
                                             scale=rgt_sb[:, h:h + 1])
                    else:
                        nc.vector.tensor_scalar_mul(ur[:, hi, :], pu, rgt_sb[:, h:h + 1])
                nc.sync.dma_start(out=urw[hg], in_=ur)

        # ------- Phase F: delta = uc+ur, int6 quantization + packing -------
        # delta carries the gamma/D scaling; residual add happens on host.
        # Per (channel, 512px-block): qv = round(delta * 30.5/amax) + 32 in
        # [1,63]; groups of 4 pixels pack into 3 bytes (little-endian 6-bit).
        with ExitStack() as ph, tc.tile_pool(name="fstage", bufs=3) as fst, \
                tc.tile_pool(name="fwork", bufs=1) as fwk, \
                tc.tile_pool(name="fpk", bufs=2) as fpkp, \
                tc.tile_pool(name="fsball", bufs=1) as fsb:
            for cc in range(4):
                cs = slice(cc * 128, (cc + 1) * 128)
                sball = fsb.tile([128, HW], bf16, tag="sball")
                for hb in range(6):
                    r0 = hb * 1536
                    uct = fst.tile([128, 1536], bf16, tag="uct")
                    nc.sync.dma_start(out=uct, in_=uc_d[r0:r0 + 1536, cs], transpose=True)
                    urt = fst.tile([128, 1536], bf16, tag="urt")
                    nc.sync.dma_start(out=urt, in_=ur_d[r0:r0 + 1536, cs], transpose=True)
                    if (cc + hb) % 2 == 0:
                        nc.gpsimd.tensor_add(sball[:, r0:r0 + 1536], uct, urt)
                    else:
                        nc.vector.tensor_add(sball[:, r0:r0 + 1536], uct, urt)
                amax = fwk.tile([128, NB], f32, tag="amax")
                for j in range(NB):
                    nc.vector.tensor_reduce(amax[:, j:j + 1], sball[:, j * 512:(j + 1) * 512],
                                            axis=mybir.AxisListType.X,
                                            op=mybir.AluOpType.max,
                                            apply_absolute_value=True)
                nc.sync.dma_start(out=outp_d[cs, PACKW:OUTW].bitcast(f32),
                                  in_=amax)
                rinv = fwk.tile([128, NB], f32, tag="rinv")
                nc.vector.reciprocal(rinv, amax)
                rs = fwk.tile([128, NB], f32, tag="rs")
                nc.scalar.activation(rs, rinv, AF.Copy, scale=QMARGIN)
                qv = fwk.tile([128, HW], u8, tag="qv")
                for j in range(NB):
                    nc.vector.tensor_scalar(qv[:, j * 512:(j + 1) * 512],
                                            sball[:, j * 512:(j + 1) * 512],
                                            rs[:, j:j + 1], 32.0,
                                            op0=OP.mult, op1=OP.add)
                # pack: w24 = v0 | v1<<6 | v2<<12 | v3<<18 -> 3 bytes
                g4 = qv.rearrange("p (n four) -> p n four", four=4)
                v0, v1 = g4[:, :, 0], g4[:, :, 1]
                v2, v3 = g4[:, :, 2], g4[:, :, 3]
                pk = fpkp.tile([128, HW // 4, 3], u8, tag="pk")
                t1 = fwk.tile([128, HW // 4], u8, tag="t1")
                nc.vector.tensor_scalar(t1, v1, 3, 6, op0=OP.bitwise_and,
                                        op1=OP.logical_shift_left)
                nc.vector.tensor_tensor(pk[:, :, 0], v0, t1, op=OP.bitwise_or)
                t2 = fwk.tile([128, HW // 4], u8, tag="t2")
                nc.vector.tensor_scalar(t2, v1, 2, None, op0=OP.logical_shift_right)
                t3 = fwk.tile([128, HW // 4], u8, tag="t3")
                nc.vector.tensor_scalar(t3, v2, 15, 4, op0=OP.bitwise_and,
                                        op1=OP.logical_shift_left)
                nc.vector.tensor_tensor(pk[:, :, 1], t2, t3, op=OP.bitwise_or)
                t4 = fwk.tile([128, HW // 4], u8, tag="t4")
                nc.vector.tensor_scalar(t4, v2, 4, None, op0=OP.logical_shift_right)
                t5 = fwk.tile([128, HW // 4], u8, tag="t5")
                nc.vector.tensor_scalar(t5, v3, 2, None, op0=OP.logical_shift_left)
                nc.vector.tensor_tensor(pk[:, :, 2], t4, t5, op=OP.bitwise_or)
                nc.sync.dma_start(out=outp_d[cs, 0:PACKW],
                                  in_=pk.rearrange("p n three -> p (n three)"))

    nc.compile()
    return nc


NGROUPS = int(__import__("os").environ.get("KERNEL_NGROUPS", "4"))
HALF_NGROUPS = int(__import__("os").environ.get("KERNEL_NGROUPS_HALF", "2"))



def _make_runner(gamma_f: float, ngroups: int = NGROUPS, lo: int = 0,
                 hi: int = NCORES):
    """Build the Bass module once and wrap it in cached jitted dispatchers
    (the axon run_bass_kernel_spmd path, minus the per-call retrace, minus
    the host-side zero-output upload). Devices [lo:hi] are split into
    `ngroups` independent dispatch groups so a later group's upload/exec
    overlaps an earlier group's download through the shared tunnel."""
    import jax
    import jax.numpy as jnp
    from jax.sharding import Mesh, PartitionSpec, NamedSharding
    try:
        from jax.experimental.shard_map import shard_map
    except ImportError:
        from jax.shard_map import shard_map
    from concourse import bass2jax, mybir
    from concourse.bass2jax import _bass_exec_p, install_neuronx_cc_hook

    nc = _build(gamma_f)
    install_neuronx_cc_hook()
    if nc.dbg_addr is not None and nc.dbg_callbacks:
        raise RuntimeError("dbg callbacks unsupported in cached dispatch")

    partition_name = nc.partition_id_tensor.name if nc.partition_id_tensor else None
    in_names, out_names, out_avals = [], [], []
    for alloc in nc.m.functions[0].allocations:
        if not isinstance(alloc, mybir.MemoryLocationSet):
            continue
        name = alloc.memorylocations[0].name
        if alloc.kind == "ExternalInput":
            if name != partition_name:
                in_names.append(name)
        elif alloc.kind == "ExternalOutput":
            out_names.append(name)
            out_avals.append(jax.core.ShapedArray(
                tuple(alloc.tensor_shape), mybir.dt.np(alloc.dtype)))
    n_params = len(in_names)
    n_outs = len(out_names)
    bind_in_names = tuple(in_names + out_names
                          + ([partition_name] if partition_name else []))

    def _body(*args):
        operands = list(args)
        if partition_name is not None:
            operands.append(bass2jax.partition_id_tensor())
        outs = _bass_exec_p.bind(
            *operands,
            out_avals=tuple(out_avals),
            in_names=bind_in_names,
            out_names=tuple(out_names),
            lowering_input_output_aliases=(),
            sim_require_finite=True,
            sim_require_nnan=True,
            nc=nc,
        )
        return tuple(outs)

    assert len(jax.devices()) >= hi, f"need {hi} devices, have {len(jax.devices())}"
    devices = jax.devices()[lo:hi]
    ndev = hi - lo
    assert ndev % ngroups == 0
    gsize = ndev // ngroups
    in_specs = (PartitionSpec("core"),) * (n_params + n_outs)
    out_specs = (PartitionSpec("core"),) * n_outs
    donate = tuple(range(n_params, n_params + n_outs))
    groups = []
    for gi in range(ngroups):
        mesh = Mesh(np.asarray(devices[gi * gsize:(gi + 1) * gsize]), ("core",))
        nshard = NamedSharding(mesh, PartitionSpec("core"))
        fn = jax.jit(
            shard_map(_body, mesh=mesh, in_specs=in_specs, out_specs=out_specs,
                      check_rep=False),
            donate_argnums=donate,
            keep_unused=True,
        )
        # Donated zero output buffers, created on-device (no host upload).
        zero_fns = [
            jax.jit(
                (lambda shape, dt: (lambda: jnp.zeros(shape, dt)))(
                    (gsize * av.shape[0],) + tuple(av.shape[1:]), av.dtype),
                out_shardings=nshard)
            for av in out_avals
        ]
        groups.append(dict(fn=fn, zero_fns=zero_fns, nshard=nshard))
    return dict(nc=nc, groups=groups, gsize=gsize, lo=lo, ndev=ndev,
                devices=devices, in_names=in_names, out_names=out_names)


_cache: dict = {}
_bufs: dict = {}

try:
    import numba

    @numba.njit(cache=True, fastmath=True, nogil=True)
    def _nb_add_amax(xsrc, dlt, xd, amax):
        # xd = xsrc + dlt[c]; amax per channel row
        for c in range(xsrc.shape[0]):
            d = dlt[c]
            row = xsrc[c]
            orow = xd[c]
            m = 1e-20
            for p in range(row.shape[0]):
                t = row[p] + d
                orow[p] = t
                a = abs(t)
                if a > m:
                    m = a
            amax[c] = m

    @numba.njit(cache=True, fastmath=True, nogil=True)
    def _nb_pack7_resid(xd, amax, xp, srow, r32):
        # int7: q = rint(x*63/amax) in [-63,63]; enc = q+64; 8 vals -> 7 bytes
        for c in range(xd.shape[0]):
            m = amax[c]
            s = 63.0 / m
            si = m / 63.0
            srow[c] = si
            row = xd[c]
            rrow = r32[c]
            prow = xp[c]
            for gset in range(row.shape[0] // 8):
                w = np.int64(0)
                base = gset * 8
                for i in range(8):
                    q = np.int64(np.rint(row[base + i] * s))
                    rrow[base + i] = row[base + i] - q * si
                    w |= (q + 64) << (7 * i)
                pb = gset * 7
                for j in range(7):
                    prow[pb + j] = np.uint8((w >> (8 * j)) & 255)

    @numba.njit(cache=True, fastmath=True, nogil=True)
    def _nb_pack_c6(cb, cp, cs2):
        # cb [128, HW] -> int6 packed rows; cs2 [64, 2]: col0 = q, col1 = k
        n = cb.shape[0] // 2
        for c in range(cb.shape[0]):
            row = cb[c]
            m = 1e-20
            for p in range(row.shape[0]):
                a = abs(row[p])
                if a > m:
                    m = a
            s = 30.5 / m
            prow = cp[c]
            for g in range(row.shape[0] // 4):
                v0 = np.int32(np.rint(row[4 * g] * s)) + 32
                v1 = np.int32(np.rint(row[4 * g + 1] * s)) + 32
                v2 = np.int32(np.rint(row[4 * g + 2] * s)) + 32
                v3 = np.int32(np.rint(row[4 * g + 3] * s)) + 32
                prow[3 * g] = np.uint8((v0 | (v1 << 6)) & 255)
                prow[3 * g + 1] = np.uint8(((v1 >> 2) | (v2 << 4)) & 255)
                prow[3 * g + 2] = np.uint8(((v2 >> 4) | (v3 << 2)) & 255)
            if c < n:
                cs2[c, 0] = m / 30.5
            else:
                cs2[c - n, 1] = m / 30.5

    @numba.njit(cache=True, fastmath=True, nogil=True)
    def _nb_unpack_dequant(acc, pk, am18):
        # acc [C, HW] += unpack6(pk [C, PACKW]) scaled by am18 [C, 18]/30.5
        for c in range(acc.shape[0]):
            prow = pk[c]
            arow = acc[c]
            for j in range(18):
                step = am18[c, j] * (1.0 / 30.5)
                g0 = j * 128  # 128 groups of 4 px per 512-block
                for gg in range(128):
                    gi = g0 + gg
                    b0 = np.int32(prow[3 * gi])
                    b1 = np.int32(prow[3 * gi + 1])
                    b2 = np.int32(prow[3 * gi + 2])
                    p0 = gi * 4
                    arow[p0] += ((b0 & 63) - 32) * step
                    arow[p0 + 1] += (((b0 >> 6) | ((b1 & 15) << 2)) - 32) * step
                    arow[p0 + 2] += (((b1 >> 4) | ((b2 & 3) << 4)) - 32) * step
                    arow[p0 + 3] += ((b2 >> 2) - 32) * step
except Exception:  # numba unavailable: numpy fallbacks below
    _nb_add_amax = None
    _nb_pack7_resid = None
    _nb_pack_c6 = None
    _nb_unpack_dequant = None


def _np_pack7_resid(xd, amax, xp, srow, r32):
    si = amax * (1.0 / 63.0)
    q = np.rint(xd * (63.0 / amax)[:, None])
    np.subtract(xd, q * si[:, None], out=r32)
    srow[:] = si
    v = (q + 64).astype(np.int64).reshape(xd.shape[0], -1, 8)
    w = np.zeros(v.shape[:2], np.int64)
    for i in range(8):
        w |= v[:, :, i] << (7 * i)
    out = np.empty(v.shape[:2] + (7,), np.uint8)
    for j in range(7):
        out[:, :, j] = (w >> (8 * j)) & 255
    xp[:] = out.reshape(xp.shape)


def _np_pack_c6(cb, cp, cs2):
    n = cb.shape[0] // 2
    amax = np.maximum(np.abs(cb).max(axis=1), 1e-20)
    v = (np.rint(cb * (30.5 / amax)[:, None]).astype(np.int32) + 32)\
        .reshape(cb.shape[0], -1, 4)
    out = np.empty(v.shape[:2] + (3,), np.uint8)
    out[:, :, 0] = (v[:, :, 0] | (v[:, :, 1] << 6)) & 255
    out[:, :, 1] = ((v[:, :, 1] >> 2) | (v[:, :, 2] << 4)) & 255
    out[:, :, 2] = ((v[:, :, 2] >> 4) | (v[:, :, 3] << 2)) & 255
    cp[:] = out.reshape(cp.shape)
    cs2[:, 0] = amax[:n] * (1.0 / 30.5)
    cs2[:, 1] = amax[n:] * (1.0 / 30.5)


def _np_unpack_dequant(acc, pk, am18):
    b = pk.reshape(acc.shape[0], -1, 3).astype(np.int32)
    v = np.empty((acc.shape[0], b.shape[1], 4), np.int32)
    v[:, :, 0] = b[:, :, 0] & 63
    v[:, :, 1] = (b[:, :, 0] >> 6) | ((b[:, :, 1] & 15) << 2)
    v[:, :, 2] = (b[:, :, 1] >> 4) | ((b[:, :, 2] & 3) << 4)
    v[:, :, 3] = b[:, :, 2] >> 2
    vals = (v - 32).reshape(acc.shape[0], 18, 512).astype(np.float32)
    vals *= (am18 * (1.0 / 30.5))[:, :, None]
    acc += vals.reshape(acc.shape)


def _prep_shared(Wq, bq, Wk, bk, Wv, bv, delta):
    Wq = np.asarray(Wq, np.float32)
    Wk = np.asarray(Wk, np.float32)
    Wv = np.asarray(Wv, np.float32)
    Wqk = np.ascontiguousarray(np.vstack([Wq, Wk]))  # (2*IC, C)
    dev = dict(
        wqkT=np.ascontiguousarray(Wqk.T).astype(np.float32).reshape(4, 128, 2 * IC),
        wvT=np.ascontiguousarray(Wv.T).astype(BF).reshape(4, 128, C),
        mwvd=(-(Wv @ delta)).astype(BF).reshape(1, C),
        ib=np.eye(96, dtype=np.float32),
        negib=np.eye(96, dtype=np.float32) * -1e30,
    )
    bqk = np.concatenate([np.asarray(bq, np.float32) - Wq @ delta,
                          np.asarray(bk, np.float32) - Wk @ delta])
    return dev, Wqk, bqk


def _getbuf(name, shape, dtype):
    b = _bufs.get(name)
    if b is None or b.shape != shape or b.dtype != dtype:
        b = _bufs[name] = np.empty(shape, dtype)
    return b


_PROF = bool(__import__("os").environ.get("KERNEL_PROF"))


def _prep_batch(xrow, delta, Wqk, bqk, xd_row, xp_row, cp_row, xs_row,
                cs_row, csn_row, r32, c32, amax):
    """One batch of host prep: residual shift, int7 pack (+residual), the
    correction sgemm on the residual, int6 pack of the correction."""
    if _nb_add_amax is not None:
        _nb_add_amax(xrow, delta, xd_row, amax)
        srow = np.empty(C, np.float32)
        _nb_pack7_resid(xd_row, amax, xp_row, srow, r32)
    else:
        np.add(xrow, delta[:, None], out=xd_row)
        np.maximum(np.abs(xd_row).max(axis=1), 1e-20, out=amax)
        srow = np.empty(C, np.float32)
        _np_pack7_resid(xd_row, amax, xp_row, srow, r32)
    xs_row[:] = srow.reshape(4, 128).T
    np.matmul(Wqk, r32, out=c32)
    c32 += bqk[:, None]
    if _nb_pack_c6 is not None:
        _nb_pack_c6(c32, cp_row, cs_row)
    else:
        _np_pack_c6(c32, cp_row, cs_row)
    csn_row[0, :IC] = -32.0 * cs_row[:, 0]
    csn_row[1, :IC] = -32.0 * cs_row[:, 1]


def _stage_group(runner, gi, xp8, cp8, xs_np, cs_np, csn_np, fresh_w, shared):
    """device_put group gi's batch planes + scales and dispatch its jitted
    fn. Batch index i within xp8 etc. maps to runner device lo+i."""
    import jax
    gr = runner["groups"][gi]
    gsize = runner["gsize"]
    b0, b1 = gi * gsize, (gi + 1) * gsize
    devices = runner["devices"]
    xparts = [jax.device_put(xp8[b], devices[b]) for b in range(b0, b1)]
    cparts = [jax.device_put(cp8[b], devices[b]) for b in range(b0, b1)]
    xg = jax.make_array_from_single_device_arrays(
        (gsize * C, XPACKW), gr["nshard"], xparts)
    cg = jax.make_array_from_single_device_arrays(
        (gsize * 2 * IC, PACKW), gr["nshard"], cparts)
    sg = jax.device_put(xs_np[b0:b1].reshape(gsize * 128, 4), gr["nshard"])
    csg = jax.device_put(cs_np[b0:b1].reshape(gsize * IC, 2), gr["nshard"])
    csng = jax.device_put(csn_np[b0:b1].reshape(gsize * 2, 512), gr["nshard"])
    per_call = {"xp": xg, "cp": cg, "xs": sg, "cs": csg, "csn": csng}
    if fresh_w:
        dev_w = {n: jax.device_put(np.concatenate([w] * gsize, axis=0),
                                   gr["nshard"])
                 for n, w in shared.items()}
        runner.setdefault("wstage", {})[gi] = dev_w
    else:
        dev_w = runner["wstage"][gi]
    args = [per_call.get(name) if name in per_call else dev_w[name]
            for name in runner["in_names"]]
    zeros = gr.pop("next_zeros", None) or [zf() for zf in gr["zero_fns"]]
    if _PROF:
        runner.setdefault("_probe", {})[gi] = (xg, cg)
    return gr["fn"](*args, *zeros)


def _weights_fresh(runner, shared):
    wc = runner.get("wcache")
    fresh = not (wc is not None
                 and all(np.array_equal(shared[n], wc[n]) for n in shared))
    if fresh:
        runner["wcache"] = {n: np.copy(w) for n, w in shared.items()}
    return fresh


def _make_fetch(runner, disp, xd_rows, pi, _mark=lambda m: None):
    gsize = runner["gsize"]

    def fetch(arg):
        gi, s = arg
        pk = np.asarray(s.data)  # (C, OUTW) u8; blocks until device done
        _mark(f"g{gi} shard dl done")
        am18 = np.ascontiguousarray(pk[:, PACKW:]).view(np.float32)\
            .reshape(C, NB)
        li = (s.index[0].start or 0) // C
        b = gi * gsize + li
        if _nb_unpack_dequant is not None:
            _nb_unpack_dequant(xd_rows[b], pk, am18)
        else:
            _np_unpack_dequant(xd_rows[b],
                               np.ascontiguousarray(pk[:, :PACKW]), am18)
    return fetch


def _run_fast(runner, x32, delta, shared, Wqk, bqk, B):
    """Single-process path: all 8 cores from this process."""
    import time as _t
    _t0 = _t.time()
    _mark = (lambda m: print(f"  [{_t.time()-_t0:7.3f}] {m}", flush=True)) \
        if _PROF else (lambda m: None)
    groups = runner["groups"]
    gsize = runner["gsize"]
    xd32 = np.empty((B, C, HW), np.float32)
    xp8 = _getbuf("xp8", (B, C, XPACKW), np.uint8)
    cp8 = _getbuf("cp8", (B, 2 * IC, PACKW), np.uint8)
    xs_np = _getbuf("xs", (B, 128, 4), np.float32)
    cs_np = _getbuf("cs", (B, IC, 2), np.float32)
    csn_np = _getbuf("csn", (B, 2, 512), np.float32)
    csn_np[:] = 0.0
    r32 = _getbuf("r32", (C, HW), np.float32)
    c32 = _getbuf("c32", (2 * IC, HW), np.float32)
    amax = _getbuf("amax", (C,), np.float32)
    fresh_w = _weights_fresh(runner, shared)
    pi = runner["out_names"].index("outp")

    disp = []
    futs = []
    fetch = _make_fetch(runner, disp, xd32, pi, _mark)
    with ThreadPoolExecutor(NCORES) as ex:
        for gi, gr in enumerate(groups):
            b0, b1 = gi * gsize, (gi + 1) * gsize
            for b in range(b0, b1):
                _prep_batch(x32[b], delta, Wqk, bqk, xd32[b], xp8[b], cp8[b],
                            xs_np[b], cs_np[b], csn_np[b], r32, c32, amax)
            disp.append(_stage_group(runner, gi, xp8, cp8, xs_np, cs_np,
                                     csn_np, fresh_w, shared))
            _mark(f"g{gi} dispatched")
            for s in disp[gi][pi].addressable_shards:
                futs.append(ex.submit(fetch, (gi, s)))
        for gi, gr in enumerate(groups):
            gr["next_zeros"] = [zf() for zf in gr["zero_fns"]]
        for f in futs:
            f.result()
        _mark("all fetched")
    return xd32


# ---------------- two-process dual-wire dispatch ----------------
# The axon tunnel caps at ~40-48MB/s per OS process but ~75-90MB/s across
# two processes, so a persistent worker process drives cores 4-7 (its own
# PJRT client = its own tunnel connection) while this process drives 0-3.
# The worker is a pure device-I/O proxy + output dequant: all prep runs on
# the (single-core) host in the main process; packed planes move through
# shared memory.

_worker: dict = {}
# Asymmetric split: the worker is a lightweight second PJRT client (its own
# tunnel connection and, crucially, its own serialized D2H stream) driving
# only the last WCORES cores; the main process drives the rest.
WCORES = int(__import__("os").environ.get("KERNEL_WCORES", "2"))
MCORES = NCORES - WCORES


def _shm_arrays(shm):
    """Views into the shared block: worker-batch inputs + xd accumulator."""
    n = WCORES
    sizes = [
        ("xp", (n, C, XPACKW), np.uint8),
        ("cp", (n, 2 * IC, PACKW), np.uint8),
        ("xs", (n, 128, 4), np.float32),
        ("cs", (n, IC, 2), np.float32),
        ("csn", (n, 2, 512), np.float32),
        ("xd", (n, C, HW), np.float32),
    ]
    out, off = {}, 0
    for name, shape, dt in sizes:
        nbytes = int(np.prod(shape)) * np.dtype(dt).itemsize
        out[name] = np.frombuffer(shm.buf, dt, int(np.prod(shape)),
                                  off).reshape(shape)
        off += nbytes
    return out, off


def _shm_total():
    n = WCORES
    return (n * C * XPACKW + n * 2 * IC * PACKW
            + (n * 128 * 4 + n * IC * 2 + n * 2 * 512 + n * C * HW) * 4)


def _worker_entry(conn, shm_name):
    import traceback
    from multiprocessing import shared_memory
    shm = shared_memory.SharedMemory(name=shm_name)
    arr, _ = _shm_arrays(shm)
    prof = bool(__import__("os").environ.get("KERNEL_PROF"))
    runner = None
    gkey = None
    shared = None
    fresh_w = True
    state = {}
    ex = ThreadPoolExecutor(NCORES)
    try:
        while True:
            msg = conn.recv()
            tag = msg[0]
            if tag == "weights":
                gnew, shared = msg[1], msg[2]
                if runner is None or gkey != gnew:
                    runner = _make_runner(gnew, ngroups=WCORES, lo=MCORES, hi=NCORES)
                    gkey = gnew
                runner.pop("wstage", None)
                fresh_w = True
                conn.send(("ready",))
            elif tag == "start":
                import time as _t
                _wt0 = _t.time()
                state = dict(disp=[], futs=[])
                state["fetch"] = _make_fetch(
                    runner, state["disp"], arr["xd"],
                    runner["out_names"].index("outp"),
                    (lambda m: print(f"  [w {_t.time()-_wt0:6.3f}] {m}",
                                     flush=True)) if prof
                    else (lambda m: None))
            elif tag == "batch":
                i = msg[1]
                gsize = runner["gsize"]
                if (i + 1) % gsize == 0:
                    gi = i // gsize
                    disp = state["disp"]
                    disp.append(_stage_group(runner, gi, arr["xp"], arr["cp"],
                                             arr["xs"], arr["cs"], arr["csn"],
                                             fresh_w, shared))
                    pi = runner["out_names"].index("outp")
                    for s in disp[gi][pi].addressable_shards:
                        state["futs"].append(ex.submit(state["fetch"], (gi, s)))
                    if gi == len(runner["groups"]) - 1:
                        fresh_w = False
                        for gr in runner["groups"]:
                            gr["next_zeros"] = [zf() for zf in gr["zero_fns"]]
                        for f in state["futs"]:
                            f.result()
                        conn.send(("done",))
            elif tag == "exit":
                break
    except BaseException:
        try:
            conn.send(("err", traceback.format_exc()))
        except Exception:
            pass
    finally:
        ex.shutdown(wait=False)


def _ensure_worker():
    if _worker.get("proc") is not None and _worker["proc"].is_alive():
        return _worker
    import multiprocessing as mp
    from multiprocessing import shared_memory
    import os, sys
    sys.path.insert(0, os.path.dirname(os.path.abspath(__file__)))
    ctx = mp.get_context("spawn")
    shm = shared_memory.SharedMemory(create=True, size=_shm_total())
    conn, child = ctx.Pipe()
    proc = ctx.Process(target=_worker_entry, args=(child, shm.name),
                       daemon=True)
    # The spawn child re-resolves the interpreter symlink and loses the nix
    # env's site-packages during early boot, which breaks the axon PJRT
    # plugin registration (sitecustomize needs numpy/jax).  Export the env's
    # site-packages via PYTHONPATH so the child boots identically.
    # Append (not prepend): the env site-packages carries the generic nix
    # sitecustomize.py which must NOT shadow the axon one on the path head.
    sp = os.path.dirname(os.path.dirname(np.__file__))
    old_pp = os.environ.get("PYTHONPATH")
    os.environ["PYTHONPATH"] = ((old_pp + ":") if old_pp else "") + sp
    try:
        proc.start()
    finally:
        if old_pp is None:
            os.environ.pop("PYTHONPATH", None)
        else:
            os.environ["PYTHONPATH"] = old_pp
    arr, _ = _shm_arrays(shm)
    _worker.update(proc=proc, conn=conn, shm=shm, arr=arr, winit=False)
    return _worker


def _worker_recv(w, want, timeout):
    if not w["conn"].poll(timeout):
        raise TimeoutError(f"worker: no '{want}' within {timeout}s")
    msg = w["conn"].recv()
    if msg[0] == "err":
        raise RuntimeError(f"worker error:\n{msg[1]}")
    if msg[0] != want:
        raise RuntimeError(f"worker: expected {want}, got {msg[0]}")
    return msg


def _run_dual(runner, x32, delta, shared, Wqk, bqk, B, g):
    import time as _t
    _t0 = _t.time()
    _mark = (lambda m: print(f"  [{_t.time()-_t0:7.3f}] {m}", flush=True)) \
        if _PROF else (lambda m: None)
    w = _ensure_worker()
    arr = w["arr"]
    first = not w["winit"]
    fresh_w = _weights_fresh(runner, shared)
    if first or fresh_w or w.get("wg") != g:
        w["conn"].send(("weights", g, shared))
        _worker_recv(w, "ready", 600 if first else 120)
        w["winit"] = True
        w["wg"] = g
        _mark("worker ready")

    res = np.empty((B, C, HW), np.float32)
    n = MCORES
    xp8 = _getbuf("xp8", (n, C, XPACKW), np.uint8)
    cp8 = _getbuf("cp8", (n, 2 * IC, PACKW), np.uint8)
    xs_np = _getbuf("xs", (n, 128, 4), np.float32)
    cs_np = _getbuf("cs", (n, IC, 2), np.float32)
    csn_np = _getbuf("csn", (n, 2, 512), np.float32)
    csn_np[:] = 0.0
    r32 = _getbuf("r32", (C, HW), np.float32)
    c32 = _getbuf("c32", (2 * IC, HW), np.float32)
    amax = _getbuf("amax", (C,), np.float32)
    pi = runner["out_names"].index("outp")
    gsize = runner["gsize"]

    w["conn"].send(("start",))
    disp = []
    futs = []
    fetch = _make_fetch(runner, disp, res, pi, _mark)
    # worker batches go early so its (small) chain hides under main's
    plan = (["w", "m", "m"] * WCORES + ["m"] * (MCORES - 2 * WCORES))[:NCORES]
    wi = 0
    mi = 0
    with ThreadPoolExecutor(4 * NCORES) as ex:
        for step in plan:
            if step == "w":
                _prep_batch(x32[MCORES + wi], delta, Wqk, bqk, arr["xd"][wi],
                            arr["xp"][wi], arr["cp"][wi], arr["xs"][wi],
                            arr["cs"][wi], arr["csn"][wi], r32, c32, amax)
                w["conn"].send(("batch", wi))
                _mark(f"worker batch {wi} sent")
                wi += 1
                continue
            b = mi
            _prep_batch(x32[b], delta, Wqk, bqk, res[b], xp8[b], cp8[b],
                        xs_np[b], cs_np[b], csn_np[b], r32, c32, amax)
            mi += 1
            if mi % gsize == 0:
                gi = mi // gsize - 1
                disp.append(_stage_group(runner, gi, xp8, cp8, xs_np, cs_np,
                                         csn_np, fresh_w, shared))
                _mark(f"main g{gi} dispatched")
                if _PROF:
                    futs.append(ex.submit(
                        (lambda gg: lambda: (disp[gg][pi].block_until_ready(),
                                             _mark(f"main g{gg} exec done"))[-1])(gi)))
                for s in disp[gi][pi].addressable_shards:
                    futs.append(ex.submit(fetch, (gi, s)))
        for gr in runner["groups"]:
            gr["next_zeros"] = [zf() for zf in gr["zero_fns"]]
        for f in futs:
            f.result()
        _mark("main part fetched")
        _worker_recv(w, "done", 120)
        _mark("worker done")
    res[MCORES:] = arr["xd"]
    return res


def _run_fallback(nc, x32, delta, shared, Wqk, bqk, B):
    from concourse.bass_utils import run_bass_kernel_spmd
    xd32 = x32 + delta[None, :, None]
    in_maps = []
    for b in range(B):
        xb = xd32[b]
        amax = np.maximum(np.abs(xb).max(axis=1), 1e-20)
        xp8 = np.empty((C, XPACKW), np.uint8)
        cp8 = np.empty((2 * IC, PACKW), np.uint8)
        srow = np.empty(C, np.float32)
        cs2 = np.empty((IC, 2), np.float32)
        r32 = np.empty((C, HW), np.float32)
        _np_pack7_resid(xb, amax, xp8, srow, r32)
        c32 = Wqk @ r32 + bqk[:, None]
        _np_pack_c6(c32, cp8, cs2)
        csn = np.zeros((2, 512), np.float32)
        csn[0, :IC] = -32.0 * cs2[:, 0]
        csn[1, :IC] = -32.0 * cs2[:, 1]
        in_maps.append(dict(
            shared,
            xp=xp8,
            xs=np.ascontiguousarray(srow.reshape(4, 128).T),
            cp=cp8,
            cs=cs2,
            csn=csn,
        ))
    res = run_bass_kernel_spmd(nc, in_maps, core_ids=list(range(B)))
    for b in range(B):
        pk = res.results[b]["outp"]
        am18 = np.ascontiguousarray(pk[:, PACKW:]).view(np.float32)\
            .reshape(C, NB)
        _np_unpack_dequant(xd32[b], np.ascontiguousarray(pk[:, :PACKW]),
                           am18)
    return xd32


def kernel(x, Wq, bq, Wk, bk, Wv, bv, gamma):
    x = np.asarray(x)
    B = x.shape[0]
    assert B == NCORES, f"expected B={NCORES}, got {B}"
    g = float(np.asarray(gamma).reshape(-1)[0])
    delta = (g * np.asarray(bv, np.float64)).astype(np.float32)
    x32 = np.asarray(x, np.float32).reshape(B, C, HW)
    shared, Wqk, bqk = _prep_shared(Wq, bq, Wk, bk, Wv, bv, delta)

    import os, sys, time, traceback
    globals()["_last_exec_ns"] = None
    globals()["_last_trace"] = None

    # Dual-process dispatch (a second PJRT client in a worker process owns
    # cores 4-7) raises the wire ceiling in pure-transfer benchmarks, but
    # with the single host CPU core the two clients' transfer pumps and
    # the prep/dequant work contend and it measures slightly slower than
    # the single-process path end-to-end; opt in with KERNEL_DUAL=1.
    import os as _os
    if _os.environ.get("KERNEL_DUAL") and _worker.get("dual_ok", True):
        try:
            key = ("main", MCORES, round(g, 9))
            if key not in _cache:
                _cache[key] = _make_runner(g, ngroups=MCORES // 2,
                                           lo=0, hi=MCORES)
            res = _run_dual(_cache[key], x32, delta, shared, Wqk, bqk, B, g)
            return res.reshape(B, C, H, W)
        except Exception:
            traceback.print_exc()
            if os.environ.get("KERNEL_NO_FALLBACK"):
                raise
            print("kernel: dual dispatch failed; single-process fallback",
                  file=sys.stderr)
            _worker["dual_ok"] = False
            try:
                if _worker.get("proc") is not None:
                    _worker["proc"].terminate()
            except Exception:
                pass

    key = ("full", round(g, 9))
    try:
        if key not in _cache:
            _cache[key] = _make_runner(g)
        res = _run_fast(_cache[key], x32, delta, shared, Wqk, bqk, B)
    except Exception:
        traceback.print_exc()
        if os.environ.get("KERNEL_NO_FALLBACK"):
            raise
        # A wedged NeuronCore (NRT_EXEC_UNIT_UNRECOVERABLE) persists for the
        # life of the PJRT client: tear the backend down, rebuild the runner
        # (terminal-side reconnect resets the cores), and retry once.
        print("kernel: fast dispatch failed; resetting backend", file=sys.stderr)
        try:
            import jax._src.xla_bridge as _xb
            _xb._clear_backends()
            import jax
            jax.clear_caches()
            time.sleep(2.0)
            _cache.clear()
            _cache[key] = runner = _make_runner(g)
            res = _run_fast(runner, x32, delta, shared, Wqk, bqk, B)
        except Exception:
            traceback.print_exc()
            print("kernel: retry failed; final fallback", file=sys.stderr)
            res = _run_fallback(_cache[key]["nc"], x32, delta, shared,
                                Wqk, bqk, B)
    return res.reshape(B, C, H, W)
